# revision 27
# baseline (speedup 1.0000x reference)
"""Causal multi-head attention on 8 trn2 NeuronCores (Megatron-style head parallelism).

Problem: B=2, L=2048, D=1024, H=16 heads (HD=64), fp32 in/out.

Sharding: each of the 8 cores owns 2 heads (a 128-wide slice of the QKV
projection output / Wo rows). Every core reads the full x; QKV projections are
column-sharded, attention runs per-head, the output projection is row-sharded
producing a partial sum per core which the host reduces (+ bo).

On-chip layout: activations are feature-major: x^T [D, B*L] (host
pre-transposes), Q^T/K^T/V^T [128(d), L] per batch. Scores are computed
transposed: S^T[k, q] = K_blk^T.T @ Q^T (contraction over head dim), exp on
the scalar engine, ctx^T[d, q] accumulates over key blocks with V-natural
(built via DMA-XBAR transpose into contiguous tiles) as the stationary
operand.

Perf structure (v2 — single interleaved PE stream):
  - everything is emitted as one continuous PE instruction stream per batch:
    attention steps carry "filler" work units (QKV projection / output
    projection matmul groups) injected between the score matmul of step i+1
    and the ctx matmul of step i.  The PE never waits on the scalar engine's
    exp (which covers its ~1us latency under the filler), stays p-state
    ramped, and the projection phases cost no extra wall-clock.
  - all PSUM work units allocate from the same 2-buf "sc" pool ring
    ([128,1024]f32 = 2 banks each; sc 4 banks + ctx 4 banks = 8 banks).
  - causal work trimmed at 128-col granularity on diagonal blocks; causal
    mask applied additively in PSUM via an identity-stationary matmul
    (value -1000 before the 1/8 softmax scale -> exp underflows to exact 0).
  - ctx packed [128d, L]: h0 ctx rows 0-64 of psum bank A (inline ones column
    gives the h0 softmax denominator in row 64), h1 ctx rows 64-127 of bank B
    with its denominator from a 1-col side-matmul into bank B row 32.
    Reciprocals via the fast DVE approx; the per-column broadcast is a rank-1
    f32r matmul into the ctx tile's free psum regions, emitted two score
    tiles into the next query tile so the PE never waits on the reciprocal.
  - output projection contracts all 128 dims in one matmul per (128-token
    block, half); interleaved into the SAME batch's attention one query tile
    behind the norm, so the tail is only the last query tile's outproj.
  - first x strip DMA'd in [64, 512] chunks ordered (half, ec, row) so the
    first projection matmul starts as soon as ~64KB has landed; weight DMAs
    split per 2 ec chunks.
"""

import numpy as np

_B, _L, _D, _H, _HD = 2, 2048, 1024, 16, 64
_NC = 8
_DC = _D // _NC          # 128 feature dims (2 heads) per core
_T = _B * _L             # 4096 tokens
_NKB = _L // 128         # 16 key blocks per batch
_NQT = _L // 512         # 4 query tiles per batch

_cache = {}


def _build_bass():
    from concourse import bacc
    import concourse.mybir as mybir
    import concourse.tile as tile

    f32 = mybir.dt.float32
    f16 = mybir.dt.float16
    bf16 = mybir.dt.bfloat16
    AFT = mybir.ActivationFunctionType

    nc = bacc.Bacc("TRN2", target_bir_lowering=False, debug=False, num_devices=_NC)

    # Every dma_start costs ~640ns of serial descriptor-generation time on
    # the sync sequencer, so host inputs are packed into few tensors:
    #   wqkv: V/K/Q weights pre-rearranged [128, 3, 8, 128] (v,k,q order)
    #   bqkv: biases [128, 3] f32 (v,k,q)
    #   cst:  [128, 336] f16 = idn | msk | ons | onr(bf16 bits)
    xT = nc.dram_tensor("xT", [_D, _T], f16, kind="ExternalInput")
    wqkvd = nc.dram_tensor("wqkv", [128, 3, 8, 128], f16, kind="ExternalInput")
    wod = nc.dram_tensor("wo", [_DC, _D], f16, kind="ExternalInput")
    bqkvd = nc.dram_tensor("bqkv", [128, 3], f32, kind="ExternalInput")
    cstd = nc.dram_tensor("cst", [128, 336], f16, kind="ExternalInput")
    out = nc.dram_tensor("out", [_T, _D], f16, kind="ExternalOutput")

    with tile.TileContext(nc) as tc:
        with (
            tc.tile_pool(name="const", bufs=1) as constp,
            tc.tile_pool(name="xs", bufs=2) as xsp,
            tc.tile_pool(name="qkv", bufs=2) as qkvp,
            tc.tile_pool(name="pr", bufs=3) as prp,
            tc.tile_pool(name="nrm", bufs=2) as nrmp,
            tc.tile_pool(name="og", bufs=4) as ogp,
            tc.tile_pool(name="sc", bufs=2, space="PSUM") as scp,  # 2x[128,1024]f32 = 4 banks
            tc.tile_pool(name="cx", bufs=2, space="PSUM") as cxp,  # 2x[128,1024]f32 = 4 banks
        ):
            # ---- persistent constants ----
            # DMA emission order tracks first-use time; one dma_start per
            # logical group to keep the serial descriptor-gen cost low.
            wqkv_sb = constp.tile([128, 3, 8, 128], f16, tag="wqkv")
            bqkv_sb = constp.tile([128, 3], f32, tag="bqkv")
            cst_sb = constp.tile([128, 336], f16, tag="cst")
            wv_sb, wk_sb, wq_sb = (wqkv_sb[:, i] for i in range(3))
            bv_sb, bk_sb, bq_sb = (bqkv_sb[:, i:i + 1] for i in range(3))
            idn_sb = cst_sb[:, 0:128]
            msk_sb = cst_sb[:, 128:256]
            ons_sb = cst_sb[:, 256:272]
            onr_sb = cst_sb[:, 272:336].bitcast(bf16)

            # wv + bias first (0.25MB), then strip-0 x in paced chunks so
            # the first V matmul starts after ~256KB, then the rest
            nc.sync.dma_start(wqkv_sb[:, 0], wqkvd[:, 0])
            nc.sync.dma_start(bqkv_sb[:], bqkvd[:])

            def prefetch_strip(xs, b, tb2, fine=False):
                c0 = b * _L + tb2 * 1024
                if fine:
                    # (half, ec-pair) chunks pace the first V-proj matmuls
                    for half in range(2):
                        for i in range(4):
                            ec0 = i * 2
                            nc.sync.dma_start(
                                xs[:, ec0:ec0 + 2, half * 512:(half + 1) * 512],
                                xT[ec0 * 128:(ec0 + 2) * 128,
                                   c0 + half * 512:c0 + (half + 1) * 512]
                                .rearrange("(c p) t -> p c t", p=128),
                            )
                else:
                    for i in range(2):
                        ec0 = i * 4
                        nc.sync.dma_start(
                            xs[:, ec0:ec0 + 4, :],
                            xT[ec0 * 128:(ec0 + 4) * 128,
                               c0:c0 + 1024].rearrange(
                                "(c p) t -> p c t", p=128
                            ),
                        )

            xs00 = xsp.tile([128, 8, 1024], f16, tag="xs", name="xs")
            prefetch_strip(xs00, 0, 0, fine=True)
            nc.sync.dma_start(wqkv_sb[:, 1:3], wqkvd[:, 1:3])
            cst_sb_dma = lambda: nc.sync.dma_start(cst_sb[:], cstd[:])
            wo_sb = constp.tile([128, 1024], f16, tag="wo")

            # persistent V stationaries. v0 = [V0 | ones]: ctx rows 0-63 +
            # h0 denom row 64. v1e = [0..0 | ones@32 | 0..0 | V1]: one fused
            # matmul yields h1 denom at row 32 and ctx at rows 64-127.
            # Constant-column init is emitted after the strip-0 units so the
            # DVE queue head doesn't block on the cst DMA.
            v0 = qkvp.tile([128, _NKB, 65], f16, tag="v0", name="v0", bufs=1)
            v1e = qkvp.tile([128, _NKB, 128], f16, tag="v1e", name="v1e", bufs=1)

            def init_v_consts():
                nc.vector.tensor_copy(v0[:, :, 64], ons_sb[:])
                nc.vector.memset(v1e[:, :, 0:64], 0.0)
                nc.vector.tensor_copy(v1e[:, :, 32], ons_sb[:])

            tls = {}

            def alloc_batch(b):
                qT = qkvp.tile([128, _L], f16, tag="qT", name="qT")
                kT = qkvp.tile([128, _L], f16, tag="kT", name="kT")
                vT = qkvp.tile([128, _L], f16, tag="vT", name="vT", bufs=1)
                ctx = qkvp.tile([128, _L], f16, tag="ctx", name="ctx")
                # V natural via DMA XBAR transpose (contiguous dests only),
                # then DVE re-pack into the strided stationary tiles.
                v0t = qkvp.tile([128, _NKB, 64], f16, tag="v0t", name="v0t", bufs=1)
                v1t = qkvp.tile([128, _NKB, 64], f16, tag="v1t", name="v1t", bufs=1)
                if b == 0:
                    # strip 1's prefetch is emitted after the strip-0 units
                    xss = [xs00, xsp.tile([128, 8, 1024], f16, tag="xs", name="xs")]
                else:
                    xss = []
                    for tb2 in range(2):
                        xs = xsp.tile([128, 8, 1024], f16, tag="xs", name="xs")
                        prefetch_strip(xs, b, tb2)
                        xss.append(xs)
                tls[b] = dict(qT=qT, kT=kT, vT=vT, ctx=ctx, v0t=v0t, v1t=v1t,
                              xss=xss)
                return tls[b]

            def emit_proj_unit(b, tb2, w_sb, b_sb, dkey, half, is_v=False):
                """Half a strip x one projection: 8 matmuls + bias drain.
                The V transpose/repack chain rides on half 1 (whole strip).
                ~1.7us of PE work."""
                tl = tls[b]
                xs = tl["xss"][tb2]
                dst = tl[dkey]
                col = half * 512
                ps = scp.tile([128, 512], f32, tag="sc", name="pps")
                for ec in range(8):
                    nc.tensor.matmul(
                        ps[:],
                        w_sb[:, ec, :],
                        xs[:, ec, col:col + 512],
                        start=(ec == 0),
                        stop=(ec == 7),
                    )
                bcol = tb2 * 1024 + col
                nc.vector.tensor_scalar_add(
                    dst[:, bcol:bcol + 512], ps[:], b_sb[:]
                )
                if is_v and half == 1:
                    v0t, v1t = tl["v0t"], tl["v1t"]
                    cs = slice(tb2 * 1024, (tb2 + 1) * 1024)
                    kbs = slice(tb2 * 8, (tb2 + 1) * 8)
                    nc.sync.dma_start(
                        v0t[:, kbs, :], dst[0:64, cs], transpose=True
                    )
                    nc.sync.dma_start(
                        v1t[:, kbs, :], dst[64:128, cs], transpose=True
                    )
                    nc.vector.tensor_copy(v0[:, kbs, 0:64], v0t[:, kbs, :])
                    nc.vector.tensor_copy(v1e[:, kbs, 64:128], v1t[:, kbs, :])

            def emit_outproj_unit(b, tkb, eng):
                """Output projection for one 128-token block: 2 matmuls +
                staging copy + DMA. ~0.45us of PE work."""
                ctx = tls[b]["ctx"]
                op = scp.tile([128, 1024], f32, tag="sc", name="op")
                for half in range(2):
                    col = half * 512
                    nc.tensor.matmul(
                        op[:, col:col + 512],
                        ctx[:, tkb * 128:(tkb + 1) * 128],
                        wo_sb[:, col:col + 512],
                        start=True, stop=True,
                    )
                stg = ogp.tile([128, 1024], f16, tag="og", name="stg")
                if eng == "scalar":
                    nc.scalar.copy(stg[:], op[:])
                else:
                    nc.vector.tensor_copy(stg[:], op[:])
                r0 = b * _L + tkb * 128
                nc.sync.dma_start(out[r0:r0 + 128, :], stg[:])

            # ---- attention machinery ----
            def emit_sc(tl, qt, kb, nk, q0, cold):
                qT, kT = tl["qT"], tl["kT"]
                j = kb - (nk - 4)
                qlo = 128 * j if j > 0 else 0
                sc = scp.tile([128, 2, 512], f32, tag="sc", name="sct")
                for h in range(2):
                    hp = h * 64
                    nc.tensor.matmul(
                        sc[:, h, qlo:512],
                        kT[hp:hp + 64, kb * 128:(kb + 1) * 128],
                        qT[hp:hp + 64, q0 + qlo:q0 + 512],
                        start=True, stop=(j < 0),
                    )
                if j >= 0:
                    for h in range(2):
                        nc.tensor.matmul(
                            sc[:, h, qlo:qlo + 128],
                            idn_sb[:],
                            msk_sb[:],
                            start=False, stop=True,
                            skip_group_check=True,
                        )
                pr = prp.tile([128, 2, 512], f16, tag="pr", name="pr")
                if kb == 0 and cold:
                    # split per head so the first ctx matmul of the
                    # query tile is not gated on both heads' exp
                    for h in range(2):
                        nc.scalar.activation(
                            pr[:, h, qlo:512], sc[:, h, qlo:512],
                            AFT.Exp, scale=0.125
                        )
                else:
                    nc.scalar.activation(
                        pr[:, :, qlo:512], sc[:, :, qlo:512],
                        AFT.Exp, scale=0.125
                    )
                return (kb, nk, qlo, pr)

            def emit_cx(info, cx):
                kb, nk, qlo, pr = info
                st = kb == 0
                sp = kb == nk - 1
                nc.tensor.matmul(
                    cx[0:65, qlo:512], v0[:, kb, :], pr[:, 0, qlo:512],
                    start=st, stop=sp, skip_group_check=True,
                )
                nc.tensor.matmul(
                    cx[0:128, 512 + qlo:1024], v1e[:, kb, :], pr[:, 1, qlo:512],
                    start=st, stop=sp, skip_group_check=True,
                )

            def make_norm(tl, cx, q0):
                ctx = tl["ctx"]

                def norm(act_bc=False, c0=0, cw=512):
                    # denoms: h0 at cx[64, bankA], h1 at cx[32, bankB].
                    # reciprocal_approx_fast only works on full-width
                    # offset-0 tiles, so stage the two denom rows into
                    # an sbuf tile and reciprocate the whole tile
                    # (garbage rows are never read).
                    rci = nrmp.tile([128, cw], f32, tag="rci", name="rci")
                    nc.vector.tensor_copy(rci[64:65, :], cx[64:65, c0:c0 + cw])
                    nc.vector.tensor_copy(rci[32:33, :],
                                          cx[32:33, 512 + c0:512 + c0 + cw])
                    rc = nrmp.tile([128, cw], f32, tag="rc", name="rc")
                    nc.vector.reciprocal_approx_fast(rc[:], rci[:])
                    # bf16 view of rc's truncated high half-words:
                    # f32 bits[31:16] == bf16 round-toward-zero
                    rcb = rc.bitcast(bf16).rearrange(
                        "p (a two) -> p a two", two=2
                    )[:, :, 1]
                    # rank-1 broadcast into free psum rows of cx
                    nc.tensor.matmul(
                        cx[0:64, 512 + c0:512 + c0 + cw],
                        onr_sb[64:65, :], rcb[64:65, :],
                        start=True, stop=True, skip_group_check=True,
                    )
                    nc.tensor.matmul(
                        cx[64:128, c0:c0 + cw], onr_sb[32:33, :], rcb[32:33, :],
                        start=True, stop=True, skip_group_check=True,
                    )
                    # DVE can read only ONE psum operand per instruction, so
                    # stage the broadcast region to sbuf, then multiply
                    bc = nrmp.tile([128, cw], f32, tag="bc", name="bc")
                    if act_bc:
                        # flushes outside attention: ACT has no exp backlog
                        nc.scalar.copy(bc[0:64, :], cx[0:64, 512 + c0:512 + c0 + cw])
                        nc.scalar.copy(bc[64:128, :], cx[64:128, c0:c0 + cw])
                    else:
                        nc.vector.tensor_copy(bc[0:64, :],
                                              cx[0:64, 512 + c0:512 + c0 + cw])
                        nc.vector.tensor_copy(bc[64:128, :],
                                              cx[64:128, c0:c0 + cw])
                    nc.vector.tensor_mul(
                        ctx[0:64, q0 + c0:q0 + c0 + cw],
                        cx[0:64, c0:c0 + cw], bc[0:64, :]
                    )
                    nc.vector.tensor_mul(
                        ctx[64:128, q0 + c0:q0 + c0 + cw],
                        cx[64:128, 512 + c0:512 + c0 + cw], bc[64:128, :]
                    )
                return norm

            def emit_attn_batch(b, tl, fillers, pend):
                """All 4 query tiles of batch b as one flat (qt, kb) stream.
                fillers: {qt: [(min_kb, closure), ...]} — one popped per step
                between sc(i) and cx(i-1); all remaining drained on the last
                step of the qt."""
                steps = [(qt, kb) for qt in range(_NQT)
                         for kb in range(4 * (qt + 1))]
                fillers = {qt: sorted(fq, key=lambda t: t[0])
                           for qt, fq in fillers.items()}
                cxs = {}
                prev = None
                for qt, kb in steps:
                    nk = 4 * (qt + 1)
                    q0 = qt * 512
                    if kb == 0:
                        cxs[qt] = cxp.tile([128, 1024], f32, tag="cx", name="cx")
                    info = emit_sc(tl, qt, kb, nk, q0, b == 0 and qt == 0)
                    fq = fillers.get(qt)
                    if fq:
                        if kb == nk - 1:
                            while fq:
                                fq.pop(0)[1]()
                        elif fq[0][0] <= kb:
                            fq.pop(0)[1]()
                    if prev is not None:
                        pinfo, pqt = prev
                        emit_cx(pinfo, cxs[pqt])
                        if pinfo[0] == pinfo[1] - 1:  # last kb: qt finished
                            pend[0] = make_norm(tl, cxs[pqt], pqt * 512)
                    prev = (info, qt)
                    if kb == 2 and pend[0] is not None:
                        pend[0]()
                        pend[0] = None
                emit_cx(prev[0], cxs[prev[1]])
                pend[0] = make_norm(tl, cxs[prev[1]], prev[1] * 512)

            # ---- schedule ----
            alloc_batch(0)
            for w_sb, b_sb, dkey, isv in ((wv_sb, bv_sb, "vT", True),
                                          (wk_sb, bk_sb, "kT", False),
                                          (wq_sb, bq_sb, "qT", False)):
                for half in range(2):
                    emit_proj_unit(0, 0, w_sb, b_sb, dkey, half, is_v=isv)
                if isv:
                    # emit after the V unit: strip-1 x, remaining consts,
                    # and the v0/v1e constant-column init (DVE)
                    prefetch_strip(tls[0]["xss"][1], 0, 1)
                    cst_sb_dma()
                    nc.sync.dma_start(wo_sb[:], wod[:])
                    init_v_consts()

            # Outproj placement rule: tg_i's norm flushes at (qt_{i+1}, kb2)
            # and its DVE chain settles ~3us later, so tg_i units go one
            # full query tile later still.
            def op_unit(b, tkb):
                return lambda: emit_outproj_unit(
                    b, tkb, "scalar" if tkb % 2 == 0 else "vector")

            def proj_fillers(b, tb2):
                u = []
                for w_sb, b_sb, dkey, isv in ((wv_sb, bv_sb, "vT", True),
                                              (wk_sb, bk_sb, "kT", False),
                                              (wq_sb, bq_sb, "qT", False)):
                    for half in range(2):
                        u.append(lambda b=b, t=tb2, w=w_sb, bb=b_sb, d=dkey, \
                                 h=half, v=isv: emit_proj_unit(
                                     b, t, w, bb, d, h, is_v=v))
                return u

            pend = [None]
            for b in range(_B):
                F = {0: [], 1: [], 2: [], 3: []}
                # strip 1 of this batch feeds qt2/qt3: V halves fill qt0
                # (v0 repack latency cover), K/Q fill early qt1
                units = proj_fillers(b, 1)
                F[0].append((1, units[0]))
                F[0].append((2, units[1]))
                for i, u in enumerate(units[2:]):
                    F[1].append((i, u))
                if b > 0:
                    # previous batch's tg2 (norms long settled) and tg3
                    # (norm flushed at qt0 kb2, settled by qt1)
                    F[0].append((1, op_unit(b - 1, 8)))
                    F[0].append((2, op_unit(b - 1, 9)))
                    F[2].append((2, op_unit(b - 1, 10)))
                    F[2].append((3, op_unit(b - 1, 11)))
                    F[1].append((6, op_unit(b - 1, 12)))
                    F[1].append((6, op_unit(b - 1, 13)))
                    F[2].append((4, op_unit(b - 1, 14)))
                    F[2].append((5, op_unit(b - 1, 15)))
                for i, tkb in enumerate(range(0, 4)):
                    F[2].append((7 + i, op_unit(b, tkb)))
                for i, tkb in enumerate(range(4, 8)):
                    F[3].append((3 + i, op_unit(b, tkb)))
                if b + 1 < _B:
                    F[2].append((1, lambda b=b: alloc_batch(b + 1)))
                    # next batch strip 0; min_kb>=9 keeps the v0/v1e repack
                    # (attached to the V half-1 unit) after this batch's
                    # qt3 ctx reads of key blocks 0-8
                    for i, u in enumerate(proj_fillers(b + 1, 0)):
                        F[3].append((10 + i, u))
                else:
                    # cover the last (unfilled, ACT-bound) steps of qt3
                    for kb, tkb in zip((11, 13, 14, 15), range(8, 12)):
                        F[3].append((kb, op_unit(b, tkb)))
                emit_attn_batch(b, tls[b], F, pend)

            # tail: last batch's final norm split into two 256-col chains,
            # each unblocking its two outproj token blocks
            pend[0](True, c0=0, cw=256)
            emit_outproj_unit(_B - 1, 12, "vector")
            pend[0](True, c0=256, cw=256)
            emit_outproj_unit(_B - 1, 13, "scalar")
            emit_outproj_unit(_B - 1, 14, "vector")
            emit_outproj_unit(_B - 1, 15, "scalar")
            pend[0] = None

    nc.compile()
    return nc


def _get_nc():
    if "nc" not in _cache:
        _cache["nc"] = _build_bass()
    return _cache["nc"]


def _host_inputs(x, Wq, bq, Wk, bk, Wv, bv, Wo, bo):
    import ml_dtypes

    x = np.asarray(x, np.float32)
    xT = np.ascontiguousarray(x.reshape(_T, _D).T.astype(np.float16))

    # cst = idn | msk | ons | onr(bf16 bits)
    kk = np.arange(128)[:, None]
    cc = np.arange(128)[None, :]
    cst = np.zeros((128, 336), np.float16)
    cst[:, 0:128] = np.eye(128, dtype=np.float16)
    cst[:, 128:256] = np.where(kk <= cc, 0.0, -1000.0).astype(np.float16)
    cst[:, 256:272] = 1.0
    cst[:, 272:336] = np.ones((128, 64), ml_dtypes.bfloat16).view(np.float16)

    in_maps = []
    for c in range(_NC):
        s = slice(c * _DC, (c + 1) * _DC)
        wqkv = np.stack([
            np.asarray(W, np.float32)[:, s].astype(np.float16)
            .reshape(8, 128, 128).transpose(1, 0, 2)
            for W in (Wv, Wk, Wq)
        ], axis=1)
        bqkv = np.stack([
            np.asarray(bb, np.float32)[s] for bb in (bv, bk, bq)
        ], axis=1)
        in_maps.append({
            "xT": xT,
            "wqkv": np.ascontiguousarray(wqkv),
            "wo": np.ascontiguousarray(np.asarray(Wo, np.float32)[s, :].astype(np.float16)),
            "bqkv": np.ascontiguousarray(bqkv),
            "cst": cst,
        })
    return in_maps


def kernel_run(x, Wq, bq, Wk, bk, Wv, bv, Wo, bo, trace=False):
    """Run the SPMD kernel; returns (full output, BassKernelResults)."""
    from concourse.bass_utils import run_bass_kernel_spmd

    nc = _get_nc()
    in_maps = _host_inputs(x, Wq, bq, Wk, bk, Wv, bv, Wo, bo)
    res = run_bass_kernel_spmd(nc, in_maps, list(range(_NC)), trace=trace)
    acc = np.zeros((_T, _D), np.float32)
    for c in range(_NC):
        acc += res.results[c]["out"]
    acc += np.asarray(bo, np.float32)[None, :]
    return acc.reshape(_B, _L, _D), res


def kernel(x, Wq, bq, Wk, bk, Wv, bv, Wo, bo):
    out, _ = kernel_run(x, Wq, bq, Wk, bk, Wv, bv, Wo, bo, trace=False)
    return out


# revision 28
# speedup vs baseline: 1.0220x; 1.0220x over previous
"""Causal multi-head attention on 8 trn2 NeuronCores (Megatron-style head parallelism).

Problem: B=2, L=2048, D=1024, H=16 heads (HD=64), fp32 in/out.

Sharding: each of the 8 cores owns 2 heads (a 128-wide slice of the QKV
projection output / Wo rows). Every core reads the full x; QKV projections are
column-sharded, attention runs per-head, the output projection is row-sharded
producing a partial sum per core which the host reduces (+ bo).

On-chip layout: activations are feature-major: x^T [D, B*L] (host
pre-transposes), Q^T/K^T/V^T [128(d), L] per batch. Scores are computed
transposed: S^T[k, q] = K_blk^T.T @ Q^T (contraction over head dim), exp on
the scalar engine, ctx^T[d, q] accumulates over key blocks with V-natural
(built via DMA-XBAR transpose into contiguous tiles) as the stationary
operand.

Perf structure (v2 — single interleaved PE stream):
  - everything is emitted as one continuous PE instruction stream per batch:
    attention steps carry "filler" work units (QKV projection / output
    projection matmul groups) injected between the score matmul of step i+1
    and the ctx matmul of step i.  The PE never waits on the scalar engine's
    exp (which covers its ~1us latency under the filler), stays p-state
    ramped, and the projection phases cost no extra wall-clock.
  - all PSUM work units allocate from the same 2-buf "sc" pool ring
    ([128,1024]f32 = 2 banks each; sc 4 banks + ctx 4 banks = 8 banks).
  - causal work trimmed at 128-col granularity on diagonal blocks; causal
    mask applied additively in PSUM via an identity-stationary matmul
    (value -1000 before the 1/8 softmax scale -> exp underflows to exact 0).
  - ctx packed [128d, L]: h0 ctx rows 0-64 of psum bank A (inline ones column
    gives the h0 softmax denominator in row 64), h1 ctx rows 64-127 of bank B
    with its denominator from a 1-col side-matmul into bank B row 32.
    Reciprocals via the fast DVE approx; the per-column broadcast is a rank-1
    f32r matmul into the ctx tile's free psum regions, emitted two score
    tiles into the next query tile so the PE never waits on the reciprocal.
  - output projection contracts all 128 dims in one matmul per (128-token
    block, half); interleaved into the SAME batch's attention one query tile
    behind the norm, so the tail is only the last query tile's outproj.
  - first x strip DMA'd in [64, 512] chunks ordered (half, ec, row) so the
    first projection matmul starts as soon as ~64KB has landed; weight DMAs
    split per 2 ec chunks.
"""

import numpy as np

_B, _L, _D, _H, _HD = 2, 2048, 1024, 16, 64
_NC = 8
_DC = _D // _NC          # 128 feature dims (2 heads) per core
_T = _B * _L             # 4096 tokens
_NKB = _L // 128         # 16 key blocks per batch
_NQT = _L // 512         # 4 query tiles per batch

_cache = {}


def _build_bass():
    from concourse import bacc
    import concourse.mybir as mybir
    import concourse.tile as tile

    f32 = mybir.dt.float32
    f16 = mybir.dt.float16
    bf16 = mybir.dt.bfloat16
    AFT = mybir.ActivationFunctionType

    nc = bacc.Bacc("TRN2", target_bir_lowering=False, debug=False, num_devices=_NC)

    # Every dma_start costs ~640ns of serial descriptor-generation time on
    # the sync sequencer, so host inputs are packed into few tensors:
    #   wqkv: V/K/Q weights pre-rearranged [128, 3, 8, 128] (v,k,q order)
    #   bqkv: biases [128, 3] f32 (v,k,q)
    #   cst:  [128, 336] f16 = idn | msk | ons | onr(bf16 bits)
    xT = nc.dram_tensor("xT", [_D, _T], f16, kind="ExternalInput")
    wqkvd = nc.dram_tensor("wqkv", [128, 3, 8, 128], f16, kind="ExternalInput")
    wod = nc.dram_tensor("wo", [_DC, _D], f16, kind="ExternalInput")
    bqkvd = nc.dram_tensor("bqkv", [128, 3], f32, kind="ExternalInput")
    cstd = nc.dram_tensor("cst", [128, 336], f16, kind="ExternalInput")
    out = nc.dram_tensor("out", [_T, _D], f16, kind="ExternalOutput")

    with tile.TileContext(nc) as tc:
        with (
            tc.tile_pool(name="const", bufs=1) as constp,
            tc.tile_pool(name="xs", bufs=2) as xsp,
            tc.tile_pool(name="qkv", bufs=2) as qkvp,
            tc.tile_pool(name="pr", bufs=3) as prp,
            tc.tile_pool(name="nrm", bufs=2) as nrmp,
            tc.tile_pool(name="og", bufs=4) as ogp,
            tc.tile_pool(name="sc", bufs=2, space="PSUM") as scp,  # 2x[128,1024]f32 = 4 banks
            tc.tile_pool(name="cx", bufs=2, space="PSUM") as cxp,  # 2x[128,1024]f32 = 4 banks
        ):
            # ---- persistent constants ----
            # DMA emission order tracks first-use time; one dma_start per
            # logical group to keep the serial descriptor-gen cost low.
            wqkv_sb = constp.tile([128, 3, 8, 128], f16, tag="wqkv")
            bqkv_sb = constp.tile([128, 3], f32, tag="bqkv")
            cst_sb = constp.tile([128, 336], f16, tag="cst")
            wv_sb, wk_sb, wq_sb = (wqkv_sb[:, i] for i in range(3))
            bv_sb, bk_sb, bq_sb = (bqkv_sb[:, i:i + 1] for i in range(3))
            idn_sb = cst_sb[:, 0:128]
            msk_sb = cst_sb[:, 128:256]
            ons_sb = cst_sb[:, 256:272]
            onr_sb = cst_sb[:, 272:336].bitcast(bf16)

            # wv + bias first (0.25MB), then strip-0 x in paced chunks so
            # the first V matmul starts after ~256KB, then the rest
            nc.sync.dma_start(wqkv_sb[:, 0], wqkvd[:, 0])
            nc.sync.dma_start(bqkv_sb[:], bqkvd[:])

            def prefetch_strip(xs, b, tb2, fine=False):
                c0 = b * _L + tb2 * 1024
                if fine:
                    # (half, ec-pair) chunks pace the first V-proj matmuls
                    for half in range(2):
                        for i in range(4):
                            ec0 = i * 2
                            nc.sync.dma_start(
                                xs[:, ec0:ec0 + 2, half * 512:(half + 1) * 512],
                                xT[ec0 * 128:(ec0 + 2) * 128,
                                   c0 + half * 512:c0 + (half + 1) * 512]
                                .rearrange("(c p) t -> p c t", p=128),
                            )
                else:
                    for i in range(2):
                        ec0 = i * 4
                        nc.sync.dma_start(
                            xs[:, ec0:ec0 + 4, :],
                            xT[ec0 * 128:(ec0 + 4) * 128,
                               c0:c0 + 1024].rearrange(
                                "(c p) t -> p c t", p=128
                            ),
                        )

            xs00 = xsp.tile([128, 8, 1024], f16, tag="xs", name="xs")
            prefetch_strip(xs00, 0, 0, fine=True)
            nc.sync.dma_start(wqkv_sb[:, 1:3], wqkvd[:, 1:3])
            cst_sb_dma = lambda: nc.sync.dma_start(cst_sb[:], cstd[:])
            wo_sb = constp.tile([128, 1024], f16, tag="wo")

            # persistent V stationaries. v0 = [V0 | ones]: ctx rows 0-63 +
            # h0 denom row 64. v1e = [0..0 | ones@32 | 0..0 | V1]: one fused
            # matmul yields h1 denom at row 32 and ctx at rows 64-127.
            # Constant-column init is emitted after the strip-0 units so the
            # DVE queue head doesn't block on the cst DMA.
            v0 = qkvp.tile([128, _NKB, 65], f16, tag="v0", name="v0", bufs=1)
            v1e = qkvp.tile([128, _NKB, 128], f16, tag="v1e", name="v1e", bufs=1)

            def init_v_consts():
                nc.vector.tensor_copy(v0[:, :, 64], ons_sb[:])
                nc.vector.memset(v1e[:, :, 0:64], 0.0)
                nc.vector.tensor_copy(v1e[:, :, 32], ons_sb[:])

            tls = {}

            def alloc_batch(b):
                qT = qkvp.tile([128, _L], f16, tag="qT", name="qT")
                kT = qkvp.tile([128, _L], f16, tag="kT", name="kT")
                vT = qkvp.tile([128, _L], f16, tag="vT", name="vT", bufs=1)
                ctx = qkvp.tile([128, _L], f16, tag="ctx", name="ctx")
                # V natural via DMA XBAR transpose (contiguous dests only),
                # then DVE re-pack into the strided stationary tiles.
                v0t = qkvp.tile([128, _NKB, 64], f16, tag="v0t", name="v0t", bufs=1)
                v1t = qkvp.tile([128, _NKB, 64], f16, tag="v1t", name="v1t", bufs=1)
                if b == 0:
                    # strip 1's prefetch is emitted after the strip-0 units
                    xss = [xs00, xsp.tile([128, 8, 1024], f16, tag="xs", name="xs")]
                else:
                    xss = []
                    for tb2 in range(2):
                        xs = xsp.tile([128, 8, 1024], f16, tag="xs", name="xs")
                        prefetch_strip(xs, b, tb2)
                        xss.append(xs)
                tls[b] = dict(qT=qT, kT=kT, vT=vT, ctx=ctx, v0t=v0t, v1t=v1t,
                              xss=xss)
                return tls[b]

            def emit_proj_unit(b, tb2, w_sb, b_sb, dkey, half, is_v=False):
                """Half a strip x one projection: 8 matmuls + bias drain.
                The V transpose/repack chain rides on half 1 (whole strip).
                ~1.7us of PE work."""
                tl = tls[b]
                xs = tl["xss"][tb2]
                dst = tl[dkey]
                col = half * 512
                ps = scp.tile([128, 512], f32, tag="sc", name="pps")
                for ec in range(8):
                    nc.tensor.matmul(
                        ps[:],
                        w_sb[:, ec, :],
                        xs[:, ec, col:col + 512],
                        start=(ec == 0),
                        stop=(ec == 7),
                    )
                bcol = tb2 * 1024 + col
                nc.vector.tensor_scalar_add(
                    dst[:, bcol:bcol + 512], ps[:], b_sb[:]
                )
                if is_v and half == 1:
                    v0t, v1t = tl["v0t"], tl["v1t"]
                    cs = slice(tb2 * 1024, (tb2 + 1) * 1024)
                    kbs = slice(tb2 * 8, (tb2 + 1) * 8)
                    nc.sync.dma_start(
                        v0t[:, kbs, :], dst[0:64, cs], transpose=True
                    )
                    nc.sync.dma_start(
                        v1t[:, kbs, :], dst[64:128, cs], transpose=True
                    )
                    nc.vector.tensor_copy(v0[:, kbs, 0:64], v0t[:, kbs, :])
                    nc.vector.tensor_copy(v1e[:, kbs, 64:128], v1t[:, kbs, :])

            def emit_outproj_unit(b, tkb, eng):
                """Output projection for one 128-token block: 2 matmuls +
                staging copy + DMA. ~0.45us of PE work."""
                ctx = tls[b]["ctx"]
                op = scp.tile([128, 1024], f32, tag="sc", name="op")
                for half in range(2):
                    col = half * 512
                    nc.tensor.matmul(
                        op[:, col:col + 512],
                        ctx[:, tkb * 128:(tkb + 1) * 128],
                        wo_sb[:, col:col + 512],
                        start=True, stop=True,
                    )
                stg = ogp.tile([128, 1024], f16, tag="og", name="stg")
                if eng == "scalar":
                    nc.scalar.copy(stg[:], op[:])
                else:
                    nc.vector.tensor_copy(stg[:], op[:])
                r0 = b * _L + tkb * 128
                nc.sync.dma_start(out[r0:r0 + 128, :], stg[:])

            # ---- attention machinery ----
            def emit_sc(tl, qt, kb, nk, q0, cold):
                qT, kT = tl["qT"], tl["kT"]
                j = kb - (nk - 4)
                qlo = 128 * j if j > 0 else 0
                sc = scp.tile([128, 2, 512], f32, tag="sc", name="sct")
                for h in range(2):
                    hp = h * 64
                    nc.tensor.matmul(
                        sc[:, h, qlo:512],
                        kT[hp:hp + 64, kb * 128:(kb + 1) * 128],
                        qT[hp:hp + 64, q0 + qlo:q0 + 512],
                        start=True, stop=(j < 0),
                    )
                if j >= 0:
                    for h in range(2):
                        nc.tensor.matmul(
                            sc[:, h, qlo:qlo + 128],
                            idn_sb[:],
                            msk_sb[:],
                            start=False, stop=True,
                            skip_group_check=True,
                        )
                pr = prp.tile([128, 2, 512], f16, tag="pr", name="pr")
                if kb == 0 and cold:
                    # split per head so the first ctx matmul of the
                    # query tile is not gated on both heads' exp
                    for h in range(2):
                        nc.scalar.activation(
                            pr[:, h, qlo:512], sc[:, h, qlo:512],
                            AFT.Exp, scale=0.125
                        )
                else:
                    nc.scalar.activation(
                        pr[:, :, qlo:512], sc[:, :, qlo:512],
                        AFT.Exp, scale=0.125
                    )
                return (kb, nk, qlo, pr)

            def emit_cx(info, cx):
                kb, nk, qlo, pr = info
                st = kb == 0
                sp = kb == nk - 1
                nc.tensor.matmul(
                    cx[0:65, qlo:512], v0[:, kb, :], pr[:, 0, qlo:512],
                    start=st, stop=sp, skip_group_check=True,
                )
                nc.tensor.matmul(
                    cx[0:128, 512 + qlo:1024], v1e[:, kb, :], pr[:, 1, qlo:512],
                    start=st, stop=sp, skip_group_check=True,
                )

            def make_norm(tl, cx, q0):
                ctx = tl["ctx"]

                def norm(act_bc=False, c0=0, cw=512):
                    # denoms: h0 at cx[64, bankA], h1 at cx[32, bankB].
                    # reciprocal_approx_fast only works on full-width
                    # offset-0 tiles, so stage the two denom rows into
                    # an sbuf tile and reciprocate the whole tile
                    # (garbage rows are never read).
                    rci = nrmp.tile([128, cw], f32, tag="rci", name="rci")
                    nc.vector.tensor_copy(rci[64:65, :], cx[64:65, c0:c0 + cw])
                    nc.vector.tensor_copy(rci[32:33, :],
                                          cx[32:33, 512 + c0:512 + c0 + cw])
                    rc = nrmp.tile([128, cw], f32, tag="rc", name="rc")
                    nc.vector.reciprocal_approx_fast(rc[:], rci[:])
                    # bf16 view of rc's truncated high half-words:
                    # f32 bits[31:16] == bf16 round-toward-zero
                    rcb = rc.bitcast(bf16).rearrange(
                        "p (a two) -> p a two", two=2
                    )[:, :, 1]
                    # rank-1 broadcast into free psum rows of cx
                    nc.tensor.matmul(
                        cx[0:64, 512 + c0:512 + c0 + cw],
                        onr_sb[64:65, :], rcb[64:65, :],
                        start=True, stop=True, skip_group_check=True,
                    )
                    nc.tensor.matmul(
                        cx[64:128, c0:c0 + cw], onr_sb[32:33, :], rcb[32:33, :],
                        start=True, stop=True, skip_group_check=True,
                    )
                    # DVE can read only ONE psum operand per instruction, so
                    # stage the broadcast region to sbuf, then multiply
                    bc = nrmp.tile([128, cw], f32, tag="bc", name="bc")
                    if act_bc:
                        # flushes outside attention: ACT has no exp backlog
                        nc.scalar.copy(bc[0:64, :], cx[0:64, 512 + c0:512 + c0 + cw])
                        nc.scalar.copy(bc[64:128, :], cx[64:128, c0:c0 + cw])
                    else:
                        nc.vector.tensor_copy(bc[0:64, :],
                                              cx[0:64, 512 + c0:512 + c0 + cw])
                        nc.vector.tensor_copy(bc[64:128, :],
                                              cx[64:128, c0:c0 + cw])
                    nc.vector.tensor_mul(
                        ctx[0:64, q0 + c0:q0 + c0 + cw],
                        cx[0:64, c0:c0 + cw], bc[0:64, :]
                    )
                    nc.vector.tensor_mul(
                        ctx[64:128, q0 + c0:q0 + c0 + cw],
                        cx[64:128, 512 + c0:512 + c0 + cw], bc[64:128, :]
                    )
                return norm

            def emit_attn_batch(b, tl, fillers, pend):
                """All 4 query tiles of batch b as one flat (qt, kb) stream.
                fillers: {qt: [(min_kb, closure), ...]} — one popped per step
                between sc(i) and cx(i-1); all remaining drained on the last
                step of the qt."""
                steps = [(qt, kb) for qt in range(_NQT)
                         for kb in range(4 * (qt + 1))]
                fillers = {qt: sorted(fq, key=lambda t: t[0])
                           for qt, fq in fillers.items()}
                cxs = {}
                prev = None
                for qt, kb in steps:
                    nk = 4 * (qt + 1)
                    q0 = qt * 512
                    if kb == 0:
                        cxs[qt] = cxp.tile([128, 1024], f32, tag="cx", name="cx")
                    info = emit_sc(tl, qt, kb, nk, q0, b == 0 and qt == 0)
                    fq = fillers.get(qt)
                    if fq:
                        if kb == nk - 1:
                            while fq:
                                fq.pop(0)[1]()
                        elif fq[0][0] <= kb:
                            fq.pop(0)[1]()
                    if prev is not None:
                        pinfo, pqt = prev
                        emit_cx(pinfo, cxs[pqt])
                        if pinfo[0] == pinfo[1] - 1:  # last kb: qt finished
                            pend[0] = make_norm(tl, cxs[pqt], pqt * 512)
                    prev = (info, qt)
                    if kb == 2 and pend[0] is not None:
                        pend[0]()
                        pend[0] = None
                emit_cx(prev[0], cxs[prev[1]])
                pend[0] = make_norm(tl, cxs[prev[1]], prev[1] * 512)

            # ---- schedule ----
            alloc_batch(0)
            for w_sb, b_sb, dkey, isv in ((wv_sb, bv_sb, "vT", True),
                                          (wk_sb, bk_sb, "kT", False),
                                          (wq_sb, bq_sb, "qT", False)):
                for half in range(2):
                    emit_proj_unit(0, 0, w_sb, b_sb, dkey, half, is_v=isv)
                if isv:
                    # emit after the V unit: strip-1 x, remaining consts,
                    # and the v0/v1e constant-column init (DVE)
                    prefetch_strip(tls[0]["xss"][1], 0, 1)
                    cst_sb_dma()
                    nc.sync.dma_start(wo_sb[:], wod[:])
                    init_v_consts()

            # Outproj placement rule: tg_i's norm flushes at (qt_{i+1}, kb2)
            # and its DVE chain settles ~3us later, so tg_i units go one
            # full query tile later still.
            def op_unit(b, tkb):
                return lambda: emit_outproj_unit(
                    b, tkb, "scalar" if tkb % 4 == 0 else "vector")

            def proj_fillers(b, tb2):
                u = []
                for w_sb, b_sb, dkey, isv in ((wv_sb, bv_sb, "vT", True),
                                              (wk_sb, bk_sb, "kT", False),
                                              (wq_sb, bq_sb, "qT", False)):
                    for half in range(2):
                        u.append(lambda b=b, t=tb2, w=w_sb, bb=b_sb, d=dkey, \
                                 h=half, v=isv: emit_proj_unit(
                                     b, t, w, bb, d, h, is_v=v))
                return u

            pend = [None]
            for b in range(_B):
                F = {0: [], 1: [], 2: [], 3: []}
                # strip 1 of this batch feeds qt2/qt3: V halves fill qt0
                # (v0 repack latency cover), K/Q fill early qt1
                units = proj_fillers(b, 1)
                F[0].append((1, units[0]))
                F[0].append((2, units[1]))
                for i, u in enumerate(units[2:]):
                    F[1].append((i, u))
                if b > 0:
                    # previous batch's tg2 (norms long settled) and tg3
                    # (norm flushed at qt0 kb2, settled by qt1)
                    F[0].append((1, op_unit(b - 1, 8)))
                    F[0].append((2, op_unit(b - 1, 9)))
                    F[2].append((2, op_unit(b - 1, 10)))
                    F[2].append((3, op_unit(b - 1, 11)))
                    F[1].append((6, op_unit(b - 1, 12)))
                    F[1].append((6, op_unit(b - 1, 13)))
                    F[2].append((4, op_unit(b - 1, 14)))
                    F[2].append((5, op_unit(b - 1, 15)))
                for i, tkb in enumerate(range(0, 4)):
                    F[2].append((7 + i, op_unit(b, tkb)))
                for i, tkb in enumerate(range(4, 8)):
                    F[3].append((3 + i, op_unit(b, tkb)))
                if b + 1 < _B:
                    F[2].append((1, lambda b=b: alloc_batch(b + 1)))
                    # next batch strip 0; min_kb>=9 keeps the v0/v1e repack
                    # (attached to the V half-1 unit) after this batch's
                    # qt3 ctx reads of key blocks 0-8
                    for i, u in enumerate(proj_fillers(b + 1, 0)):
                        F[3].append((10 + i, u))
                else:
                    # cover the last (unfilled, ACT-bound) steps of qt3
                    for kb, tkb in zip((11, 13, 14, 15), range(8, 12)):
                        F[3].append((kb, op_unit(b, tkb)))
                emit_attn_batch(b, tls[b], F, pend)

            # tail: last batch's final norm split into two 256-col chains,
            # each unblocking its two outproj token blocks
            pend[0](True, c0=0, cw=256)
            emit_outproj_unit(_B - 1, 12, "vector")
            pend[0](True, c0=256, cw=256)
            emit_outproj_unit(_B - 1, 13, "scalar")
            emit_outproj_unit(_B - 1, 14, "vector")
            emit_outproj_unit(_B - 1, 15, "scalar")
            pend[0] = None

    nc.compile()
    return nc


def _get_nc():
    if "nc" not in _cache:
        _cache["nc"] = _build_bass()
    return _cache["nc"]


def _host_inputs(x, Wq, bq, Wk, bk, Wv, bv, Wo, bo):
    import ml_dtypes

    x = np.asarray(x, np.float32)
    xT = np.ascontiguousarray(x.reshape(_T, _D).T.astype(np.float16))

    # cst = idn | msk | ons | onr(bf16 bits)
    kk = np.arange(128)[:, None]
    cc = np.arange(128)[None, :]
    cst = np.zeros((128, 336), np.float16)
    cst[:, 0:128] = np.eye(128, dtype=np.float16)
    cst[:, 128:256] = np.where(kk <= cc, 0.0, -1000.0).astype(np.float16)
    cst[:, 256:272] = 1.0
    cst[:, 272:336] = np.ones((128, 64), ml_dtypes.bfloat16).view(np.float16)

    in_maps = []
    for c in range(_NC):
        s = slice(c * _DC, (c + 1) * _DC)
        wqkv = np.stack([
            np.asarray(W, np.float32)[:, s].astype(np.float16)
            .reshape(8, 128, 128).transpose(1, 0, 2)
            for W in (Wv, Wk, Wq)
        ], axis=1)
        bqkv = np.stack([
            np.asarray(bb, np.float32)[s] for bb in (bv, bk, bq)
        ], axis=1)
        in_maps.append({
            "xT": xT,
            "wqkv": np.ascontiguousarray(wqkv),
            "wo": np.ascontiguousarray(np.asarray(Wo, np.float32)[s, :].astype(np.float16)),
            "bqkv": np.ascontiguousarray(bqkv),
            "cst": cst,
        })
    return in_maps


def kernel_run(x, Wq, bq, Wk, bk, Wv, bv, Wo, bo, trace=False):
    """Run the SPMD kernel; returns (full output, BassKernelResults)."""
    from concourse.bass_utils import run_bass_kernel_spmd

    nc = _get_nc()
    in_maps = _host_inputs(x, Wq, bq, Wk, bk, Wv, bv, Wo, bo)
    res = run_bass_kernel_spmd(nc, in_maps, list(range(_NC)), trace=trace)
    acc = np.zeros((_T, _D), np.float32)
    for c in range(_NC):
        acc += res.results[c]["out"]
    acc += np.asarray(bo, np.float32)[None, :]
    return acc.reshape(_B, _L, _D), res


def kernel(x, Wq, bq, Wk, bk, Wv, bv, Wo, bo):
    out, _ = kernel_run(x, Wq, bq, Wk, bk, Wv, bv, Wo, bo, trace=False)
    return out
